# revision 12
# baseline (speedup 1.0000x reference)
"""MultiHeadAttn1D (N=4, C=256, T=2048, H=8, dk=dv=32) Trainium2 Bass kernel.

Sharding: 8 cores = 4 batches x 2 head-groups (4 heads each).
Per core: x_n [256,2048] -> q,k [128,2048] ([4h*32d, t]); vT [t,o] tiles;
scores^T per (tq-chunk, s-tile) via K=32 row-tiled matmuls -> PSUM;
exp on ScalarE PSUM->SBUF (bottleneck engine, kept saturated);
PV + ones-column sums via M=32 col-tiled matmuls accumulating in PSUM;
normalize with DVE reciprocal+mul; DMA out.
"""

import sys

if "/opt/trn_rl_repo" not in sys.path:
    sys.path.insert(0, "/opt/trn_rl_repo")

import numpy as np

N_CORES = 8
C = 256          # in channels
T = 2048         # sequence length
HG = 4           # heads per core
DK = 32          # head dim
OC = 128         # output channels per core (HG * DK)
TQ = 512         # tq chunk width (1 psum bank)
NCHUNK = T // TQ          # 4
NST = T // 128            # 16 s-tiles
INV_SQRT_DK = 1.0 / np.sqrt(DK)

USE_FP32R = True
TRACE = False
LAST = {}

_CACHE = {}


def _build_module():
    import concourse.bass as bass
    from concourse import bacc, mybir
    import concourse.tile as tile

    f32 = mybir.dt.float32
    f32r = mybir.dt.float32r if USE_FP32R else mybir.dt.float32

    nc = bacc.Bacc(
        "TRN2",
        target_bir_lowering=False,
        debug=False,
        num_devices=N_CORES,
    )

    x_d = nc.dram_tensor("x", [C, T], f32, kind="ExternalInput").ap()
    wqT_d = nc.dram_tensor("wqT", [C, OC], f32, kind="ExternalInput").ap()
    wkT_d = nc.dram_tensor("wkT", [C, OC], f32, kind="ExternalInput").ap()
    wvT_d = nc.dram_tensor("wvT", [C, OC], f32, kind="ExternalInput").ap()
    out_d = nc.dram_tensor("out", [OC, T], f32, kind="ExternalOutput").ap()

    with tile.TileContext(nc) as tc:
        import contextlib

        ctx = contextlib.ExitStack()
        with ctx:
            persist = ctx.enter_context(tc.tile_pool(name="persist", bufs=1))
            pt_pool = ctx.enter_context(tc.tile_pool(name="pt", bufs=6))
            epi_pool = ctx.enter_context(tc.tile_pool(name="epi", bufs=2))
            ps_sc = ctx.enter_context(tc.tile_pool(name="ps_sc", bufs=2, space="PSUM"))
            ps_pv = ctx.enter_context(tc.tile_pool(name="ps_pv", bufs=2, space="PSUM"))
            ps_sm = ctx.enter_context(tc.tile_pool(name="ps_sm", bufs=2, space="PSUM"))

            stage = ctx.enter_context(tc.tile_pool(name="stage", bufs=3))

            # ---------------- persistent SBUF (fp32r operands for matmuls) ----------------
            wqT_sb = persist.tile([128, C], f32r)      # c-tile j at cols [128j:128j+128] = o
            wkT_sb = persist.tile([128, C], f32r)
            wvT_sb = persist.tile([128, C], f32r)
            x_sb = persist.tile([128, 2 * T], f32r)    # c-tile j at cols [T*j : T*j+T]
            q_sb = persist.tile([128, T], f32r)        # [o=4h*32, t]
            k_sb = persist.tile([128, T], f32r)
            # vT_pad: per (h, i) a [128,128] lhsT block at cols [h*T + 128*i],
            # holding vT values in sub-cols [32h:32h+32], zeros elsewhere.
            vT_pad = persist.tile([128, HG * T], f32r)
            # ones_pad: block h at cols [128h:128h+128], ones in sub-cols [32h:32h+32].
            ones_pad = persist.tile([128, HG * 128], f32r)
            ones_f32 = persist.tile([128, HG * 128], f32)

            # ---------------- DMAs (stage fp32, round into fp32r tiles) ----------------
            for w_sb, w_d in ((wqT_sb, wqT_d), (wkT_sb, wkT_d), (wvT_sb, wvT_d)):
                for j in range(2):
                    wst = stage.tile([128, TQ], f32, tag="stage",
                                     name=f"wst_{w_sb.name}_{j}")
                    nc.sync.dma_start(wst[:, 0:128], w_d[128 * j:128 * (j + 1), :])
                    nc.vector.tensor_copy(w_sb[:, 128 * j:128 * (j + 1)], wst[:, 0:128])
            for t in range(NCHUNK):
                for j in range(2):
                    xst = stage.tile([128, TQ], f32, tag="stage", name=f"xst_{j}_{t}")
                    nc.sync.dma_start(xst[:],
                                      x_d[128 * j:128 * (j + 1), TQ * t:TQ * (t + 1)])
                    nc.vector.tensor_copy(
                        x_sb[:, T * j + TQ * t: T * j + TQ * (t + 1)], xst[:])

            nc.vector.memset(vT_pad.bitcast(mybir.dt.uint32)[:], 0)
            nc.vector.memset(ones_f32[:], 0.0)
            for h in range(HG):
                nc.vector.memset(ones_f32[:, 128 * h + 32 * h: 128 * h + 32 * (h + 1)], 1.0)
            nc.vector.tensor_copy(ones_pad[:], ones_f32[:])

            # ---------------- projection helpers ----------------
            def proj_chunk(dst_sb, w_sb, t):
                """q/k projection for t-chunk: out[o, TQ] accumulated over 2 c-tiles."""
                ps = ps_sm.tile([128, TQ], f32, tag="smallps", name=f"projps_{dst_sb.name}_{t}")
                for j in range(2):
                    nc.tensor.matmul(
                        ps[:],
                        w_sb[:, 128 * j:128 * (j + 1)],
                        x_sb[:, T * j + TQ * t: T * j + TQ * (t + 1)],
                        start=(j == 0), stop=(j == 1))
                nc.vector.tensor_copy(dst_sb[:, TQ * t:TQ * (t + 1)], ps[:])

            def proj_vT(i):
                """vT for s-tile i: out[t=128, o=128] accumulated over 2 c-tiles."""
                ps = ps_sm.tile([128, 128], f32, tag="smallps", name=f"vtps_{i}")
                for j in range(2):
                    nc.tensor.matmul(
                        ps[:],
                        x_sb[:, T * j + 128 * i: T * j + 128 * (i + 1)],
                        wvT_sb[:, 128 * j:128 * (j + 1)],
                        start=(j == 0), stop=(j == 1))
                for h in range(HG):
                    nc.vector.tensor_copy(
                        vT_pad[:, h * T + 128 * i + 32 * h: h * T + 128 * i + 32 * (h + 1)],
                        ps[:, 32 * h:32 * (h + 1)])

            # pre-loop: just enough for the first exps
            proj_chunk(q_sb, wqT_sb, 0)
            proj_chunk(k_sb, wkT_sb, 0)

            # deferred projection work, drip-fed one task per attention iter
            tasks = [
                lambda: [proj_vT(i) for i in range(0, 4)],
                lambda: [proj_vT(i) for i in range(4, 8)],
                lambda: proj_chunk(k_sb, wkT_sb, 1),
                lambda: [proj_vT(i) for i in range(8, 12)],
                lambda: [proj_vT(i) for i in range(12, 16)],
                lambda: proj_chunk(k_sb, wkT_sb, 2),
                lambda: proj_chunk(q_sb, wqT_sb, 1),
                lambda: proj_chunk(k_sb, wkT_sb, 3),
                lambda: proj_chunk(q_sb, wqT_sb, 2),
                lambda: proj_chunk(q_sb, wqT_sb, 3),
            ]
            tasks.reverse()

            # ---------------- attention ----------------
            pv_ps = {}
            sums_ps = {}
            pending = []  # (c, i, pT_A, pT_B) awaiting PV/sums emission

            def emit_pv_sums(c, i, pTs):
                first = (i == 0)
                last = (i == NST - 1)
                for h in range(HG):
                    pT = pTs[h // 2]
                    sl = pT[:, TQ * (h % 2):TQ * (h % 2 + 1)]
                    nc.tensor.matmul(
                        pv_ps[c][:],
                        vT_pad[:, h * T + 128 * i: h * T + 128 * (i + 1)],
                        sl,
                        start=(first and h == 0), stop=(last and h == HG - 1))
                for h in range(HG):
                    pT = pTs[h // 2]
                    sl = pT[:, TQ * (h % 2):TQ * (h % 2 + 1)]
                    nc.tensor.matmul(
                        sums_ps[c][:],
                        ones_pad[:, 128 * h:128 * (h + 1)],
                        sl,
                        start=(first and h == 0), stop=(last and h == HG - 1))

            def emit_epilogue(c):
                recip = epi_pool.tile([128, TQ], f32, tag="recip", name=f"recip_{c}")
                nc.vector.reciprocal(recip[:], sums_ps[c][:])
                outsb = epi_pool.tile([128, TQ], f32, tag="outsb", name=f"outsb_{c}")
                nc.vector.tensor_mul(outsb[:], pv_ps[c][:], recip[:])
                nc.sync.dma_start(out_d[:, TQ * c:TQ * (c + 1)], outsb[:])

            def flush_one():
                c0, i0, pTs = pending.pop(0)
                emit_pv_sums(c0, i0, pTs)
                if i0 == NST - 1:
                    emit_epilogue(c0)

            for c in range(NCHUNK):
                pv_ps[c] = ps_pv.tile([128, TQ], f32, tag="pv", name=f"pv_{c}")
                sums_ps[c] = ps_sm.tile([128, TQ], f32, tag="smallps", name=f"sums_{c}")
                for i in range(NST):
                    if tasks:
                        tasks.pop()()
                    pTs = []
                    for half in range(2):  # half 0: heads 0,1; half 1: heads 2,3
                        sc = ps_sc.tile([128, 2 * TQ], f32, tag="sc",
                                        name=f"sc_{c}_{i}_{half}")
                        for hh in range(2):
                            h = 2 * half + hh
                            nc.tensor.matmul(
                                sc[:, TQ * hh:TQ * (hh + 1)],
                                k_sb[32 * h:32 * (h + 1), 128 * i:128 * (i + 1)],
                                q_sb[32 * h:32 * (h + 1), TQ * c:TQ * (c + 1)],
                                start=True, stop=True,
                                tile_position=(32 * h, 0))
                        pT = pt_pool.tile([128, 2 * TQ], f32r, tag="pt",
                                          name=f"pt_{c}_{i}_{half}", bufs=6)
                        nc.scalar.activation(
                            pT[:], sc[:],
                            mybir.ActivationFunctionType.Exp,
                            scale=float(INV_SQRT_DK))
                        pTs.append(pT)
                    pending.append((c, i, pTs))
                    if len(pending) > 2:
                        flush_one()
            while pending:
                flush_one()

    nc.compile()
    return nc


def _get_module():
    if "nc" not in _CACHE:
        _CACHE["nc"] = _build_module()
    return _CACHE["nc"]


def kernel(x, Wq, Wk, Wv):
    from concourse.bass_utils import run_bass_kernel_spmd

    nc = _get_module()
    x = np.ascontiguousarray(x, dtype=np.float32)
    in_maps = []
    for core in range(N_CORES):
        n, g = divmod(core, 2)
        rows = slice(128 * g, 128 * (g + 1))
        in_maps.append({
            "x": np.ascontiguousarray(x[n]),
            "wqT": np.ascontiguousarray(Wq[rows, :].T),
            "wkT": np.ascontiguousarray(Wk[rows, :].T),
            "wvT": np.ascontiguousarray(Wv[rows, :].T),
        })
    res = run_bass_kernel_spmd(nc, in_maps, core_ids=list(range(N_CORES)), trace=TRACE)
    LAST["res"] = res
    out = np.empty((4, 256, T), dtype=np.float32)
    for core in range(N_CORES):
        n, g = divmod(core, 2)
        out[n, 128 * g:128 * (g + 1), :] = res.results[core]["out"]
    return out


if __name__ == "__main__":
    _build_module()
    print("module built OK")
